# revision 21
# baseline (speedup 1.0000x reference)
"""GAT layer kernel for Trainium2, data-parallel over batch across 8 NeuronCores.

Per batch element b (one core each):
    hp  = h @ W_proj + b_proj                      # [N, D]
    s   = hp @ w_src ; t = hp @ w_dst              # [N]
    e   = relu(s[:,None] + t[None,:] + b_att)      # [N, N]
    att = exp(e) * a ; att /= att.sum(-1, keepdim) # [N, N]
    out = att @ hp + hp                            # [N, D]

Identities:
  exp(relu(x)) == max(exp(x), 1)
  exp(s_i+t_j+b) == u_i * v_j with u = exp(s), v = exp(t + b)

v2 design (vs the 193us xbar-transpose baseline):
  - the NxN work per 128-row block r is two DVE ops:
      z_r  = max(v_full * u_r, 1)   tensor_scalar, 4x bf16 mode (~0.7us)
      pb_r = z_r * a_r              tensor_tensor, 2x bf16 mode (~1.2us)
  - P^T is produced mostly by PE is_transpose matmuls (the xbar DMA
    transpose costs the SDMA engines as much as the whole a-load, so only
    a few blocks use it); ACT copies PSUM->SBUF in 512-col groups.
  - main matmul per block: lhsT = pbT chunk (stationary), rhs = hp_aug
    [j, 129] whose last column is ones -> rowsum lands in psum col 128,
    so no accum / no reduction pass is needed.
  - out natural [i, d] comes straight from the PE (no output transpose).
  - setup avoids slow paths: h is cast-loaded bf16 FIRST (before the a
    prefetch floods the queues), h^T and hp come from two tiny xbar
    transposes, t is broadcast across partitions with a ones matmul
    instead of a log2(P) chain of SBUF->SBUF DMAs.
"""

import os
import sys

for _p in ("/opt/trn_rl_repo", "/root/.axon_site/_ro/trn_rl_repo"):
    if _p not in sys.path and os.path.isdir(_p):
        sys.path.append(_p)

import numpy as np
from contextlib import ExitStack

import concourse.bass as bass
import concourse.bacc as bacc
import concourse.tile as tile
from concourse import masks, mybir
from concourse.bass_utils import run_bass_kernel_spmd

F32 = mybir.dt.float32
BF16 = mybir.dt.bfloat16

B, N, D = 8, 2048, 128
P = 128           # partitions
NT = N // P       # 16 row/col blocks
QB = 4            # row-blocks per a-load quad
NQ = NT // QB     # 4 quads
N_CORES = 8

# blocks whose P^T transpose runs on the xbar DMA instead of the PE.
# Empty: every dma_start_transpose stalls the whole DMA pipeline (tile
# serializes it against in-flight DMAs), which bubbles the a-load stream.
XBAR_BLOCKS = ()

AF = mybir.ActivationFunctionType
ALU = mybir.AluOpType


def _build_kernel(ctx: ExitStack, tc: tile.TileContext, io: dict):
    nc = tc.nc
    a = io["a"]            # [N, N] f32 dram
    h = io["h"]            # [N, D] f32 dram
    W = io["W_proj"]       # [D, D] f32 dram
    b_proj = io["b_proj"]  # [D, 1] f32 dram
    w_sd = io["w_sd"]      # [D, 2] f32 dram: [w_src | w_dst]
    b_att = io["b_att"]    # [1, 1] f32 dram
    out = io["out"]        # [N, D] f32 dram

    cst = ctx.enter_context(tc.tile_pool(name="cst", bufs=1))
    sps = ctx.enter_context(tc.tile_pool(name="sps", bufs=1, space="PSUM"))
    a_pool = ctx.enter_context(tc.tile_pool(name="a", bufs=1))

    # ---- small consts first (their DMA-lane position must precede the h
    # halves so the bf16 casts unblock immediately), then h f32 on the
    # HWDGE ring in two halves so hT transposes start on the first ----
    W_sb = cst.tile([P, D], F32)
    nc.sync.dma_start(W_sb[:], W[:])
    bp_col = cst.tile([P, 1], F32)
    nc.sync.dma_start(bp_col[:], b_proj[:])
    wsd_sb = cst.tile([P, 2], F32)
    nc.sync.dma_start(wsd_sb[:], w_sd[:])
    ba_sb = cst.tile([1, 1], F32)
    nc.sync.dma_start(ba_sb[:], b_att[:])

    h_f32 = cst.tile([P, NT, D], F32)
    h_r = h.rearrange("(r p) d -> p r d", p=P)
    H2 = NT // 2
    nc.sync.dma_start(h_f32[:, 0:H2, :], h_r[:, 0:H2, :])
    nc.sync.dma_start(h_f32[:, H2:NT, :], h_r[:, H2:NT, :])

    # identities for PE transposes (f32 built on gpsimd, bf16 cast on DVE)
    ident_f = cst.tile([P, P], F32)
    masks.make_identity(nc, ident_f[:])
    ident = cst.tile([P, P], BF16)
    nc.vector.tensor_copy(ident[:], ident_f[:])

    # ---- a prefetch: all 4 quads issued now, each one big cast-DMA ----
    a_tiles = []
    for q in range(NQ):
        a_t = a_pool.tile([P, QB, N], BF16, tag=f"a{q}")
        nc.gpsimd.dma_start(
            a_t[:],
            a[q * QB * P:(q + 1) * QB * P, :].rearrange(
                "(u p) j -> p u j", p=P))
        a_tiles.append(a_t)

    # ---- bf16 weights ----
    W_b16 = cst.tile([P, D], BF16)
    nc.vector.tensor_copy(W_b16[:], W_sb[:])
    wsd_b16 = cst.tile([P, 2], BF16)
    nc.vector.tensor_copy(wsd_b16[:], wsd_sb[:])
    # ---- hT [d, r, p] (flat: [d, n]) via PE transposes of h blocks ----
    hT3 = cst.tile([P, NT, P], BF16)
    for g in range(4):
        tp = sps.tile([P, 512], F32, tag=f"sp{g % 2}")
        for c4 in range(4):
            r = 4 * g + c4
            nc.tensor.matmul(tp[:, c4 * P:(c4 + 1) * P], h_f32[:, r, :],
                             ident_f[:], is_transpose=True)
        nc.scalar.copy(
            hT3[:, 4 * g:4 * g + 4, :].rearrange("d a b -> d (a b)"), tp[:])
    hT = hT3[:].rearrange("d r p -> d (r p)")

    # ---- hpT [d, n] = (h @ W + b).T : lhsT=W [in,out], rhs=hT [in,n] ----
    hpT = cst.tile([P, N], BF16)
    for s4 in range(4):
        sl = slice(s4 * 512, (s4 + 1) * 512)
        ps = sps.tile([P, 512], F32, tag=f"sp{s4 % 2}")
        nc.tensor.matmul(ps[:], W_b16[:], hT[:, sl])
        nc.scalar.activation(hpT[:, sl], ps[:], AF.Identity,
                             bias=bp_col[:], scale=1.0)

    # ---- v_row [1, n] = exp(hp @ w_dst + b_att), bf16 ----
    v_row = cst.tile([1, N], BF16)
    for s4 in range(4):
        sl = slice(s4 * 512, (s4 + 1) * 512)
        ps = sps.tile([P, 512], F32, tag=f"sp{s4 % 2}")
        nc.tensor.matmul(ps[:1, :], wsd_b16[:, 1:2], hpT[:, sl])
        nc.scalar.activation(v_row[:, sl], ps[:1, :], AF.Exp,
                             bias=ba_sb[:], scale=1.0)

    # ---- u [p, r] = exp(s), s = hp @ w_src ----
    s_ps = sps.tile([P, 512], F32, tag="sp0")
    for r in range(NT):
        nc.tensor.matmul(s_ps[:, r:r + 1], hpT[:, r * P:(r + 1) * P],
                         wsd_b16[:, 0:1])
    u_sb = cst.tile([P, NT], F32)
    nc.scalar.activation(u_sb[:], s_ps[:, :NT], AF.Exp)

    # ---- v_full = v_row broadcast to 128 partitions via ones matmul ----
    ones_c = cst.tile([1, P], BF16)
    nc.vector.memset(ones_c[:], 1.0)
    v_full = cst.tile([P, N], BF16)
    for s4 in range(4):
        sl = slice(s4 * 512, (s4 + 1) * 512)
        ps = sps.tile([P, 512], F32, tag=f"sp{s4 % 2}")
        nc.tensor.matmul(ps[:], ones_c[:], v_row[:, sl])
        nc.scalar.copy(v_full[:, sl], ps[:])

    # ---- hp natural [p, r, d] via PE transposes; hp_aug adds ones col ----
    hp_t = cst.tile([P, NT, D], BF16)
    for g in range(4):
        tp = sps.tile([P, 512], BF16, tag=f"sb{g % 2}")
        for c4 in range(4):
            r = 4 * g + c4
            nc.tensor.matmul(tp[:, c4 * P:(c4 + 1) * P],
                             hpT[:, r * P:(r + 1) * P], ident[:],
                             is_transpose=True)
        nc.scalar.copy(
            hp_t[:, 4 * g:4 * g + 4, :].rearrange("p a b -> p (a b)"), tp[:])
    hp_aug = cst.tile([P, NT, 130], BF16)
    hp_res = cst.tile([P, NT, D], F32)
    nc.vector.memset(hp_aug[:, :, D:D + 1], 1.0)

    def emit_hp_copies():
        # issued after block 0's z/pb ops: keeps them off the z0 critical
        # path (first consumer is block 0's matmul group / finalize)
        nc.vector.tensor_copy(hp_aug[:, :, 0:D], hp_t[:])
        nc.vector.tensor_copy(hp_res[:], hp_t[:])

    # ---- main loop pools ----
    z_pool = ctx.enter_context(tc.tile_pool(name="z", bufs=1))
    pb_pool = ctx.enter_context(tc.tile_pool(name="pb", bufs=1))
    pbt_pool = ctx.enter_context(tc.tile_pool(name="pbt", bufs=1))
    tps_pool = ctx.enter_context(tc.tile_pool(name="tps", bufs=1, space="PSUM"))
    ops_pool = ctx.enter_context(tc.tile_pool(name="ops", bufs=1, space="PSUM"))
    rs_pool = ctx.enter_context(tc.tile_pool(name="rs", bufs=1))
    osb_pool = ctx.enter_context(tc.tile_pool(name="osb", bufs=1))
    o2_pool = ctx.enter_context(tc.tile_pool(name="o2", bufs=1))

    out_t = out.rearrange("(r p) d -> p r d", p=P)

    # finalize lagged 2 blocks so in-order ACT/DVE streams never stall on
    # the current block's matmul group
    pending = []

    def finalize(o_ps, r):
        rinv = rs_pool.tile([P, 1], F32, tag=f"ri{r % 2}")
        nc.vector.reciprocal(rinv[:], o_ps[:, D:D + 1])
        o_sb = osb_pool.tile([P, D], F32, tag=f"os{r % 3}")
        nc.scalar.activation(o_sb[:], o_ps[:, 0:D], AF.Copy, scale=rinv[:])
        o2 = o2_pool.tile([P, D], F32, tag=f"o2{r % 3}")
        nc.gpsimd.tensor_tensor(o2[:], o_sb[:], hp_res[:, r, :], ALU.add)
        nc.sync.dma_start(out_t[:, r, :], o2[:])

    for r in range(NT):
        q, uq = divmod(r, QB)
        a_view = a_tiles[q][:, uq, :]                      # [128, N] bf16

        z_t = z_pool.tile([P, N], BF16, tag=f"z{r % 3}")
        nc.vector.tensor_scalar(z_t[:], v_full[:], u_sb[:, r:r + 1], 1.0,
                                ALU.mult, ALU.max)
        pb = pb_pool.tile([P, N], BF16, tag=f"pb{r % 3}")
        nc.vector.tensor_tensor(pb[:], z_t[:], a_view, ALU.mult)

        pbT = pbt_pool.tile([P, N], BF16, tag=f"pt{r % 3}")
        if r in XBAR_BLOCKS:
            nc.sync.dma_start_transpose(
                pbT[:].rearrange("p (g i) -> p g i", g=NT), pb[:])
        else:
            for g in range(2):
                tp = tps_pool.tile([P, 8 * P], BF16, tag=f"tp{g}")
                for c8 in range(8):
                    c = 8 * g + c8
                    nc.tensor.matmul(tp[:, c8 * P:(c8 + 1) * P],
                                     pb[:, c * P:(c + 1) * P], ident[:],
                                     is_transpose=True)
                nc.scalar.copy(pbT[:, 8 * g * P:(8 * g + 8) * P], tp[:])
        if r == 0:
            emit_hp_copies()

        o_ps = ops_pool.tile([P, 132], F32, tag=f"o{r % 2}")
        for c in range(NT):
            nc.tensor.matmul(o_ps[:, 0:D + 1], pbT[:, c * P:(c + 1) * P],
                             hp_aug[:, c, 0:D + 1],
                             start=(c == 0), stop=(c == NT - 1))

        pending.append((o_ps, r))
        if len(pending) > 2:
            finalize(*pending.pop(0))

    for item in pending:
        finalize(*item)


_CACHE = {}


def _get_compiled():
    if "nc" in _CACHE:
        return _CACHE["nc"], _CACHE["names"]

    nc = bacc.Bacc("TRN2", target_bir_lowering=False, debug=False)
    io = {}
    io["a"] = nc.dram_tensor("a", [N, N], F32, kind="ExternalInput").ap()
    io["h"] = nc.dram_tensor("h", [N, D], F32, kind="ExternalInput").ap()
    io["W_proj"] = nc.dram_tensor("W_proj", [D, D], F32, kind="ExternalInput").ap()
    io["b_proj"] = nc.dram_tensor("b_proj", [D, 1], F32, kind="ExternalInput").ap()
    io["w_sd"] = nc.dram_tensor("w_sd", [D, 2], F32, kind="ExternalInput").ap()
    io["b_att"] = nc.dram_tensor("b_att", [1, 1], F32, kind="ExternalInput").ap()
    io["out"] = nc.dram_tensor("out", [N, D], F32, kind="ExternalOutput").ap()

    with tile.TileContext(nc) as tc:
        with ExitStack() as ctx:
            _build_kernel(ctx, tc, io)
    nc.compile()

    _CACHE["nc"] = nc
    _CACHE["names"] = list(io.keys())
    return nc, _CACHE["names"]


def _make_in_maps(a, h, W_proj, b_proj, w_att, b_att):
    a = np.ascontiguousarray(a, dtype=np.float32)
    h = np.ascontiguousarray(h, dtype=np.float32)
    W_proj = np.ascontiguousarray(W_proj, dtype=np.float32)
    b_proj = np.ascontiguousarray(b_proj, dtype=np.float32).reshape(D, 1)
    w_att = np.ascontiguousarray(w_att, dtype=np.float32)
    w_sd = np.stack([w_att[:D], w_att[D:]], axis=1).copy()  # [D, 2]
    b_att = np.asarray(b_att, dtype=np.float32).reshape(1, 1).copy()

    in_maps = []
    for c in range(N_CORES):
        in_maps.append({
            "a": a[c], "h": h[c], "W_proj": W_proj, "b_proj": b_proj,
            "w_sd": w_sd, "b_att": b_att,
        })
    return in_maps


def _get_executable():
    """Build (once) a sharded PJRT callable for the compiled Bass module.

    Mirrors concourse.bass2jax.run_bass_via_pjrt but keeps the jitted
    function so repeated calls don't retrace/recompile.
    """
    if "exe" in _CACHE:
        return _CACHE["exe"]

    import jax
    from jax.sharding import Mesh, PartitionSpec
    from jax.experimental.shard_map import shard_map
    from concourse import bass2jax, mybir as _mybir

    nc, _ = _get_compiled()
    bass2jax.install_neuronx_cc_hook()

    partition_name = (nc.partition_id_tensor.name
                      if nc.partition_id_tensor else None)
    in_names, out_names, out_avals, zero_outs = [], [], [], []
    for alloc in nc.m.functions[0].allocations:
        if not isinstance(alloc, _mybir.MemoryLocationSet):
            continue
        name = alloc.memorylocations[0].name
        if alloc.kind == "ExternalInput":
            if name != partition_name:
                in_names.append(name)
        elif alloc.kind == "ExternalOutput":
            shape = tuple(alloc.tensor_shape)
            dtype = _mybir.dt.np(alloc.dtype)
            out_names.append(name)
            out_avals.append(jax.core.ShapedArray(shape, dtype))
            zero_outs.append(np.zeros(shape, dtype))
    n_params = len(in_names)
    n_outs = len(out_avals)
    all_in_names = in_names + out_names + (
        [partition_name] if partition_name else [])
    donate = tuple(range(n_params, n_params + n_outs))

    def _body(*args):
        operands = list(args)
        if partition_name is not None:
            operands.append(bass2jax.partition_id_tensor())
        outs = bass2jax._bass_exec_p.bind(
            *operands,
            out_avals=tuple(out_avals),
            in_names=tuple(all_in_names),
            out_names=tuple(out_names),
            lowering_input_output_aliases=(),
            sim_require_finite=True,
            sim_require_nnan=True,
            nc=nc,
        )
        return tuple(outs)

    devices = jax.devices()[:N_CORES]
    mesh = Mesh(np.asarray(devices), ("core",))
    in_specs = (PartitionSpec("core"),) * (n_params + n_outs)
    out_specs = (PartitionSpec("core"),) * n_outs
    fn = jax.jit(
        shard_map(_body, mesh=mesh, in_specs=in_specs, out_specs=out_specs,
                  check_rep=False),
        donate_argnums=donate, keep_unused=True,
    )
    exe = {
        "fn": fn, "mesh": mesh, "in_names": in_names,
        "out_names": out_names, "out_avals": out_avals,
        "zero_outs": zero_outs, "n_params": n_params,
    }
    _CACHE["exe"] = exe
    return exe


def _concat_inputs(exe, in_maps):
    return [
        np.concatenate([np.asarray(in_maps[c][name])
                        for c in range(N_CORES)], axis=0)
        for name in exe["in_names"]
    ]


def _concat_zeros(exe):
    return [np.zeros((N_CORES * z.shape[0], *z.shape[1:]), z.dtype)
            for z in exe["zero_outs"]]


def kernel(a, h, W_proj, b_proj, w_att, b_att):
    exe = _get_executable()
    in_maps = _make_in_maps(a, h, W_proj, b_proj, w_att, b_att)
    out_arrs = exe["fn"](*_concat_inputs(exe, in_maps), *_concat_zeros(exe))
    i = exe["out_names"].index("out")
    return np.asarray(out_arrs[i]).reshape(N_CORES, N, D)


if __name__ == "__main__":
    rng = np.random.default_rng(0)
    a = rng.random((B, N, N), dtype=np.float32)
    h = rng.standard_normal((B, N, D)).astype(np.float32)
    W_proj = (rng.standard_normal((D, D)) / np.sqrt(D)).astype(np.float32)
    b_proj = (rng.standard_normal(D) * 0.01).astype(np.float32)
    w_att = (rng.standard_normal(2 * D) / np.sqrt(2 * D)).astype(np.float32)
    b_att = np.float32(rng.standard_normal() * 0.01)

    got = kernel(a=a, h=h, W_proj=W_proj, b_proj=b_proj, w_att=w_att,
                 b_att=b_att)

    hp = h @ W_proj + b_proj
    s = hp @ w_att[:D]
    t = hp @ w_att[D:]
    e = np.maximum(s[:, :, None] + t[:, None, :] + b_att, 0.0)
    att = np.exp(e) * a
    att = att / att.sum(-1, keepdims=True)
    ref = att @ hp + hp

    err = np.abs(got - ref).max() / np.abs(ref).max()
    print("rel err:", err)
